# revision 1
# baseline (speedup 1.0000x reference)
"""MoE layer (8 experts, top-2) for 8 Trainium2 NeuronCores.

Strategy: expert-parallel. The router (0.1% of FLOPs) runs on host and
decides the sharding: tokens are all-to-all'd by routed expert (host-side
gather, since kernel() holds the full inputs). Each core runs one expert's
dense MLP  y = scale * (gelu(x @ W1 + b1) @ W2 + b2)  over the tokens routed
to it, with fp32r (TF32-like full-rate) matmuls on the tensor engine.
Host scatter-adds the per-expert partial outputs back (the unshard).
"""

import os

import numpy as np

HIDDEN = 1024
FF = 2 * HIDDEN
NUM_EXPERTS = 8
TOP_K = 2
NCORES = 8

# Set by kernel() when MOE_TRACE=1: HW kernel execution time in ns.
LAST_EXEC_NS = None
LAST_RESULTS = None

_PROGRAM_CACHE = {}


def _round_up(v, m):
    return (v + m - 1) // m * m


def _build_program(C, blk):
    """Bass/Tile program for one expert MLP over C tokens (SPMD on 8 cores).

    Layouts (per core):
      xT  [HIDDEN, C] f32r  - gathered tokens, transposed
      w1  [HIDDEN, FF] f32r, b1 [FF] f32
      w2  [FF, HIDDEN] f32r, b2 [HIDDEN] f32
      scl [C] f32           - per-token combine weight
      yT  [HIDDEN, C] f32   - output, transposed

    Stage B: hT[f, t] = gelu(sum_h w1[h, f] * xT[h, t] + b1[f])  (PSUM acc over
    8 h-chunks; lhsT = w1 chunk [128h, 128f], rhs = xT chunk [128h, blk])
    Stage C: yT[o, t] = (sum_f w2[f, o] * hT[f, t] + b2[o]) * scl[t]
    """
    import concourse.bass as bass  # noqa: F401
    import concourse.mybir as mybir
    import concourse.tile as tile
    from concourse import bacc

    HC = HIDDEN // 128  # 8 h-chunks
    FC = FF // 128  # 16 f-chunks
    f32 = mybir.dt.float32
    f32r = mybir.dt.float32r

    nc = bacc.Bacc("TRN2", target_bir_lowering=False, debug=False,
                   num_devices=NCORES)
    W1G, W2G = 16, 8  # dma chunk counts for w1 (along f) and w2 (along fc)
    FPER = FC // W1G  # f-chunks per w1 group
    CPER = FC // W2G  # f-chunks per w2 group
    # w1p/w2p are host-packed in SBUF tile layout (partition-major per
    # group) so every weight DMA reads >=4KB contiguous per partition.
    xT = nc.dram_tensor("xT", [HIDDEN, C], f32r, kind="ExternalInput")
    w1 = nc.dram_tensor(
        "w1p", [128, W1G, HC, FPER * 128], f32r, kind="ExternalInput")
    b1 = nc.dram_tensor("b1", [FF], f32, kind="ExternalInput")
    w2 = nc.dram_tensor(
        "w2p", [128, W2G, CPER, HIDDEN], f32r, kind="ExternalInput")
    b2 = nc.dram_tensor("b2", [HIDDEN], f32, kind="ExternalInput")
    scl = nc.dram_tensor("scl", [C], f32, kind="ExternalInput")
    yT = nc.dram_tensor("yT", [HIDDEN, C], f32, kind="ExternalOutput")

    # Token blocks: fp32r needs moving dim >= 256 for full PE rate, so split
    # the ragged tail into two >=256 pieces instead of leaving a small block.
    blocks = []
    t0 = 0
    rem = C
    while rem > 0:
        if rem >= blk + 256 or rem <= blk:
            b = min(blk, rem)
        else:
            b = rem - 256
        if b < 256 and rem > b:
            b = rem
        blocks.append((t0, b))
        t0 += b
        rem -= b

    Gelu = mybir.ActivationFunctionType.Gelu
    Ident = mybir.ActivationFunctionType.Identity

    with tile.TileContext(nc) as tc:
        with (
            tc.tile_pool(name="wts", bufs=1) as wts,
            tc.tile_pool(name="xin", bufs=2) as xin,
            tc.tile_pool(name="sin", bufs=3) as sin,
            tc.tile_pool(name="hmid", bufs=1) as hmid,
            tc.tile_pool(name="outs", bufs=3) as outs,
            tc.tile_pool(name="ps", bufs=4, space="PSUM") as ps,
        ):
            # --- resident weights/biases ---
            # Weights are loaded in chunks (separate tiles) so the first
            # matmuls only wait for the first slice, and the rest of the
            # ~17MB streams in underneath compute.
            fper, cper = FPER, CPER

            def in_ring():
                return nc.sync

            def emit_xs(t0, bs):
                # two half-tiles (4 h-chunks each): fewer DMAs in the
                # critical startup prefix (issue time ~0.65us each)
                xcs = []
                for half in range(2):
                    xc = xin.tile([128, 4, blk], f32r, tag="x", name=f"xh{half}")
                    nc.sync.dma_start(
                        out=xc[:, :, :bs],
                        in_=xT.ap().rearrange(
                            "(c p) t -> p c t", p=128)[
                            :, half * 4:(half + 1) * 4, t0:t0 + bs])
                    xcs.append(xc)
                s_sb = sin.tile([128, blk], f32, tag="s", name="s")
                nc.sync.dma_start(
                    out=s_sb[:, :bs],
                    in_=scl.ap()[t0:t0 + bs].partition_broadcast(128))
                return xcs, s_sb

            # Ring order: w1g0 first (the first matmul's weights), then
            # block 0's activations, then the rest of the weight stream
            # (w1 before w2 — consumption order), all HWDGE.
            w1_g = []
            w2_g = []
            for g in range(1):
                t = wts.tile([128, HC, fper * 128], f32r, tag=f"w1g{g}")
                in_ring().dma_start(out=t[:], in_=w1.ap()[:, g])
                w1_g.append(t)
            pre_x = {blocks[0]: emit_xs(*blocks[0])}
            for g in range(1, W1G):
                t = wts.tile([128, HC, fper * 128], f32r, tag=f"w1g{g}")
                in_ring().dma_start(out=t[:], in_=w1.ap()[:, g])
                w1_g.append(t)
            b1_sb = wts.tile([128, FC], f32)
            nc.scalar.dma_start(
                out=b1_sb[:], in_=b1.ap().rearrange("(c p) -> p c", p=128))
            b2_sb = wts.tile([128, HC], f32)
            nc.scalar.dma_start(
                out=b2_sb[:], in_=b2.ap().rearrange("(c p) -> p c", p=128))
            for g in range(W2G):
                t = wts.tile([128, cper, HIDDEN], f32r, tag=f"w2g{g}")
                in_ring().dma_start(out=t[:], in_=w2.ap()[:, g])
                w2_g.append(t)

            def w1_lhsT(hc, fc):
                return w1_g[fc // fper][:, hc, (fc % fper) * 128:(fc % fper + 1) * 128]

            def w2_lhsT(fc, oc):
                return w2_g[fc // cper][:, fc % cper, oc * 128:(oc + 1) * 128]

            for t0, bs in blocks:
                if (t0, bs) in pre_x:
                    x_sb, s_sb = pre_x[(t0, bs)]
                else:
                    x_sb, s_sb = emit_xs(t0, bs)

                h_sb = hmid.tile([128, FC, blk], f32r, tag="h")
                for fc in range(FC):
                    ph = ps.tile([128, blk], f32, tag="ps")
                    for hc in range(HC):
                        nc.tensor.matmul(
                            ph[:, :bs],
                            w1_lhsT(hc, fc),
                            x_sb[hc // 4][:, hc % 4, :bs],
                            start=(hc == 0), stop=(hc == HC - 1),
                        )
                    nc.scalar.activation(
                        out=h_sb[:, fc, :bs], in_=ph[:, :bs],
                        func=Gelu, bias=b1_sb[:, fc:fc + 1], scale=1.0)

                for oc in range(HC):
                    py = ps.tile([128, blk], f32, tag="ps")
                    for fc in range(FC):
                        nc.tensor.matmul(
                            py[:, :bs],
                            w2_lhsT(fc, oc),
                            h_sb[:, fc, :bs],
                            start=(fc == 0), stop=(fc == FC - 1),
                        )
                    o1 = outs.tile([128, blk], f32, tag="o1")
                    nc.scalar.activation(
                        out=o1[:, :bs], in_=py[:, :bs], func=Ident,
                        bias=b2_sb[:, oc:oc + 1], scale=1.0)
                    nc.vector.tensor_mul(o1[:, :bs], o1[:, :bs], s_sb[:, :bs])
                    nc.scalar.dma_start(
                        out=yT.ap().rearrange(
                            "(c p) t -> p c t", p=128)[:, oc, t0:t0 + bs],
                        in_=o1[:, :bs])

    nc.compile()
    return nc


def _route_host(x, Wr, br):
    """Replicate the reference router bit-exactly (jax on CPU), with a
    numpy fallback (same math, same tie semantics) if jax-cpu is absent."""
    try:
        import jax
        import jax.numpy as jnp

        cpu = jax.devices("cpu")[0]
        xj = jax.device_put(x, cpu)
        Wrj = jax.device_put(Wr, cpu)
        brj = jax.device_put(br, cpu)
        with jax.default_device(cpu):
            logits = jnp.einsum("bsh,he->bse", xj, Wrj) + brj
            routing = jax.nn.softmax(logits, axis=-1)
            topw, topi = jax.lax.top_k(routing, TOP_K)
            topw = jax.nn.softmax(topw, axis=-1)
        return np.asarray(topw), np.asarray(topi)
    except Exception:
        lg = x.reshape(-1, x.shape[-1]).astype(np.float32) @ Wr + br
        m = lg.max(axis=-1, keepdims=True)
        p = np.exp(lg - m)
        p /= p.sum(axis=-1, keepdims=True)
        # top-k with lower-index-wins tie semantics (jax.lax.top_k)
        topi = np.argsort(-p, axis=-1, kind="stable")[:, :TOP_K]
        topv = np.take_along_axis(p, topi, axis=-1)
        e = np.exp(topv - topv.max(axis=-1, keepdims=True))
        topw = (e / e.sum(axis=-1, keepdims=True)).astype(np.float32)
        B, S = x.shape[0], x.shape[1]
        return (topw.reshape(B, S, TOP_K),
                topi.astype(np.int32).reshape(B, S, TOP_K))


def kernel(x, Wr, br, W1, b1, W2, b2):
    global LAST_EXEC_NS, LAST_RESULTS
    from concourse.bass_utils import run_bass_kernel_spmd

    x = np.ascontiguousarray(np.asarray(x, dtype=np.float32))
    Wr = np.asarray(Wr, dtype=np.float32)
    br = np.asarray(br, dtype=np.float32)
    W1 = np.ascontiguousarray(np.asarray(W1, dtype=np.float32))
    b1 = np.ascontiguousarray(np.asarray(b1, dtype=np.float32))
    W2 = np.ascontiguousarray(np.asarray(W2, dtype=np.float32))
    b2 = np.ascontiguousarray(np.asarray(b2, dtype=np.float32))

    B, S, H = x.shape
    ntok = B * S
    xf = x.reshape(ntok, H)

    topw, topi = _route_host(x, Wr, br)
    topw = topw.reshape(ntok, TOP_K)
    topi = topi.reshape(ntok, TOP_K)

    # per-expert token index lists + combine weights
    idx = []
    wgt = []
    for e in range(NUM_EXPERTS):
        mask = (topi == e)
        tok = np.nonzero(mask.any(axis=1))[0]
        w = (topw * mask).sum(axis=1)[tok].astype(np.float32)
        idx.append(tok)
        wgt.append(w)
    counts = np.array([len(t) for t in idx])

    blk = int(os.environ.get("MOE_BLK", "512"))
    C = max(_round_up(int(counts.max()), 2), 512)

    key = (C, blk)
    if key not in _PROGRAM_CACHE:
        _PROGRAM_CACHE[key] = _build_program(C, blk)
    nc = _PROGRAM_CACHE[key]

    in_maps = []
    for e in range(NUM_EXPERTS):
        xTe = np.zeros((H, C), dtype=np.float32)
        xTe[:, :counts[e]] = xf[idx[e]].T
        scle = np.zeros((C,), dtype=np.float32)
        scle[:counts[e]] = wgt[e]
        # pack weights into the kernel's SBUF tile layout:
        # w1p[p, g, c, fg] = W1[e][c*128+p, g*FPER*128+fg]
        W1G, W2G = 16, 8
        HC, FC = H // 128, 2 * H // 128
        FPER, CPER = FC // W1G, FC // W2G
        w1p = np.ascontiguousarray(
            W1[e].reshape(HC, 128, W1G, FPER * 128).transpose(1, 2, 0, 3))
        # w2p[p, g, cc, h] = W2[e][(g*CPER+cc)*128+p, h]
        w2p = np.ascontiguousarray(
            W2[e].reshape(W2G, CPER, 128, H).transpose(2, 0, 1, 3))
        in_maps.append({
            "xT": xTe,
            "w1p": w1p,
            "b1": np.ascontiguousarray(b1[e]),
            "w2p": w2p,
            "b2": np.ascontiguousarray(b2[e]),
            "scl": scle,
        })

    trace = os.environ.get("MOE_TRACE", "0") == "1"
    res = run_bass_kernel_spmd(
        nc, in_maps, core_ids=list(range(NCORES)), trace=trace)
    LAST_EXEC_NS = res.exec_time_ns
    LAST_RESULTS = res

    out = np.zeros((ntok, H), dtype=np.float32)
    for e in range(NUM_EXPERTS):
        ye = res.results[e]["yT"][:, :counts[e]].T  # [cnt, H]
        out[idx[e]] += ye
    return out.reshape(B, S, H)



# revision 3
# speedup vs baseline: 1.0536x; 1.0536x over previous
"""MoE layer (8 experts, top-2) for 8 Trainium2 NeuronCores.

Strategy: expert-parallel. The router (0.1% of FLOPs) runs on host and
decides the sharding: tokens are gathered by routed expert host-side (the
all-to-all), each core runs one expert's dense MLP
  y = scale * (gelu(x @ W1 + b1) @ W2 + b2)
over the tokens routed to it, and the host scatter-adds the per-expert
partial outputs back.

Kernel structure (per core): all tensors bf16 on the wire (PSUM accumulates
f32), weight-stationary passes over the full token range. Tokens are split
into NB <= 8 blocks of <= 512 (one PSUM bank each). Stage B runs 16
f-chunk passes (8 k-steps x NB blocks, one LDWEIGHTS-worth of weights per
k-step held across all blocks); stage C runs 8 output-chunk passes (16
k-steps x NB blocks). Weight/x DMAs are ordered so the first matmul only
waits on ~0.8MB.
"""

import os

import numpy as np

HIDDEN = 1024
FF = 2 * HIDDEN
NUM_EXPERTS = 8
TOP_K = 2
NCORES = 8
HC = HIDDEN // 128   # 8 k-chunks for stage B / out-chunks for stage C
FC = FF // 128       # 16 f-chunks

# Set by kernel() when MOE_TRACE=1: HW kernel execution time in ns.
LAST_EXEC_NS = None
LAST_RESULTS = None

_PROGRAM_CACHE = {}


def _round_up(v, m):
    return (v + m - 1) // m * m


def _blocks_of(C, blk):
    """Split C tokens into blocks of <=blk, avoiding tiny tails."""
    blocks = []
    t0 = 0
    rem = C
    while rem > 0:
        if rem >= blk + 256 or rem <= blk:
            b = min(blk, rem)
        else:
            b = rem - 256
        if b < 256 and rem > b:
            b = rem
        blocks.append((t0, b))
        t0 += b
        rem -= b
    return blocks


def _build_program(C, blk):
    import concourse.bass as bass  # noqa: F401
    import concourse.mybir as mybir
    import concourse.tile as tile
    from concourse import bacc

    f32 = mybir.dt.float32
    bf16 = mybir.dt.bfloat16

    nc = bacc.Bacc("TRN2", target_bir_lowering=False, debug=False,
                   num_devices=NCORES)

    # DRAM layouts (host-packed, partition-major so every DMA reads >=2KB
    # contiguous per partition):
    #   xp [128, HC, C]       bf16: xp[p, hc, t] = x_t[hc*128+p]
    #   w1p[128, FC, HC, 128] bf16: w1p[p, fc, hc, j] = W1[hc*128+p, fc*128+j]
    #   w2p[128, OC, FC, 128] bf16: w2p[p, oc, fc, j] = W2[fc*128+p, oc*128+j]
    #   yT [128, OC, C]       f32:  y_t[oc*128+p] = yT[p, oc, t]
    xp = nc.dram_tensor("xp", [128, HC, C], bf16, kind="ExternalInput")
    w1 = nc.dram_tensor("w1p", [128, FC, HC, 128], bf16, kind="ExternalInput")
    b1 = nc.dram_tensor("b1", [FF], f32, kind="ExternalInput")
    w2 = nc.dram_tensor("w2p", [128, HC, FC, 128], bf16, kind="ExternalInput")
    b2 = nc.dram_tensor("b2", [HIDDEN], f32, kind="ExternalInput")
    scl = nc.dram_tensor("scl", [C], f32, kind="ExternalInput")
    yT = nc.dram_tensor("yT", [128, HC, C], f32, kind="ExternalOutput")

    blocks = _blocks_of(C, blk)
    NB = len(blocks)
    assert NB <= 8, f"need one PSUM bank per block, got {NB}"

    Gelu = mybir.ActivationFunctionType.Gelu
    Ident = mybir.ActivationFunctionType.Identity

    with tile.TileContext(nc) as tc:
        with (
            tc.tile_pool(name="wts", bufs=1) as wts,
            tc.tile_pool(name="xin", bufs=1) as xin,
            tc.tile_pool(name="hmid", bufs=1) as hmid,
            tc.tile_pool(name="outs", bufs=4) as outs,
            tc.tile_pool(name="ps", bufs=8, space="PSUM") as ps,
        ):
            # --- DMA issue order on the sync ring: first matmul's deps
            # first (w1[fc=0], x[hc=0]), then the rest of x (all passes
            # need all of x), then remaining w1, then w2 (needed ~halfway).
            w1_sb = [None] * FC
            x_sb = [None] * HC

            def load_w1(fc):
                t = wts.tile([128, HC, 128], bf16, tag=f"w1f{fc}", name=f"w1f{fc}")
                nc.sync.dma_start(out=t[:], in_=w1.ap()[:, fc])
                w1_sb[fc] = t

            def load_x(hc):
                t = xin.tile([128, C], bf16, tag=f"x{hc}", name=f"x{hc}")
                nc.sync.dma_start(out=t[:], in_=xp.ap()[:, hc])
                x_sb[hc] = t

            load_w1(0)
            for hc in range(HC):
                load_x(hc)
            for fc in range(1, FC):
                load_w1(fc)
            w2_sb = []
            for oc in range(HC):
                t = wts.tile([128, FC, 128], bf16, tag=f"w2o{oc}", name=f"w2o{oc}")
                nc.sync.dma_start(out=t[:], in_=w2.ap()[:, oc])
                w2_sb.append(t)

            # small stuff on the scalar ring
            b1_sb = wts.tile([128, FC], f32, tag="b1")
            nc.scalar.dma_start(
                out=b1_sb[:], in_=b1.ap().rearrange("(c p) -> p c", p=128))
            b2_sb = wts.tile([128, HC], f32, tag="b2")
            nc.scalar.dma_start(
                out=b2_sb[:], in_=b2.ap().rearrange("(c p) -> p c", p=128))
            s_sb = wts.tile([128, C], f32, tag="scl")
            nc.scalar.dma_start(
                out=s_sb[:], in_=scl.ap().partition_broadcast(128))

            h_sb = [hmid.tile([128, C], bf16, tag=f"h{fc}", name=f"h{fc}")
                    for fc in range(FC)]

            # --- Stage B: h[fc] = gelu(sum_hc w1[hc,fc].T @ x[hc] + b1[fc])
            for fc in range(FC):
                phs = [ps.tile([128, blk], f32, tag="ps", name=f"psB{fc}_{b}")
                       for b in range(NB)]
                for hc in range(HC):
                    lhsT = w1_sb[fc][:, hc]
                    for b, (t0, bs) in enumerate(blocks):
                        nc.tensor.matmul(
                            phs[b][:, :bs],
                            lhsT,
                            x_sb[hc][:, t0:t0 + bs],
                            start=(hc == 0), stop=(hc == HC - 1),
                        )
                for b, (t0, bs) in enumerate(blocks):
                    nc.scalar.activation(
                        out=h_sb[fc][:, t0:t0 + bs], in_=phs[b][:, :bs],
                        func=Gelu, bias=b1_sb[:, fc:fc + 1], scale=1.0)

            # --- Stage C: y[oc] = scl * (sum_fc w2[fc,oc].T @ h[fc] + b2[oc])
            for oc in range(HC):
                pys = [ps.tile([128, blk], f32, tag="ps", name=f"psC{oc}_{b}")
                       for b in range(NB)]
                for fc in range(FC):
                    lhsT = w2_sb[oc][:, fc]
                    for b, (t0, bs) in enumerate(blocks):
                        nc.tensor.matmul(
                            pys[b][:, :bs],
                            lhsT,
                            h_sb[fc][:, t0:t0 + bs],
                            start=(fc == 0), stop=(fc == FC - 1),
                        )
                for b, (t0, bs) in enumerate(blocks):
                    o1 = outs.tile([128, blk], f32, tag="o1", name=f"o{oc}_{b}")
                    nc.scalar.activation(
                        out=o1[:, :bs], in_=pys[b][:, :bs], func=Ident,
                        bias=b2_sb[:, oc:oc + 1], scale=1.0)
                    nc.vector.tensor_mul(
                        o1[:, :bs], o1[:, :bs], s_sb[:, t0:t0 + bs])
                    nc.scalar.dma_start(
                        out=yT.ap()[:, oc, t0:t0 + bs], in_=o1[:, :bs])

    nc.compile()
    return nc


def _route_host(x, Wr, br):
    """Replicate the reference router bit-exactly (jax on CPU), with a
    numpy fallback (same math, same tie semantics) if jax-cpu is absent."""
    try:
        import jax
        import jax.numpy as jnp

        cpu = jax.devices("cpu")[0]
        xj = jax.device_put(x, cpu)
        Wrj = jax.device_put(Wr, cpu)
        brj = jax.device_put(br, cpu)
        with jax.default_device(cpu):
            logits = jnp.einsum("bsh,he->bse", xj, Wrj) + brj
            routing = jax.nn.softmax(logits, axis=-1)
            topw, topi = jax.lax.top_k(routing, TOP_K)
            topw = jax.nn.softmax(topw, axis=-1)
        return np.asarray(topw), np.asarray(topi)
    except Exception:
        lg = x.reshape(-1, x.shape[-1]).astype(np.float32) @ Wr + br
        m = lg.max(axis=-1, keepdims=True)
        p = np.exp(lg - m)
        p /= p.sum(axis=-1, keepdims=True)
        # top-k with lower-index-wins tie semantics (jax.lax.top_k)
        topi = np.argsort(-p, axis=-1, kind="stable")[:, :TOP_K]
        topv = np.take_along_axis(p, topi, axis=-1)
        e = np.exp(topv - topv.max(axis=-1, keepdims=True))
        topw = (e / e.sum(axis=-1, keepdims=True)).astype(np.float32)
        B, S = x.shape[0], x.shape[1]
        return (topw.reshape(B, S, TOP_K),
                topi.astype(np.int32).reshape(B, S, TOP_K))


def kernel(x, Wr, br, W1, b1, W2, b2):
    global LAST_EXEC_NS, LAST_RESULTS
    import ml_dtypes
    from concourse.bass_utils import run_bass_kernel_spmd

    bf16 = ml_dtypes.bfloat16

    x = np.ascontiguousarray(np.asarray(x, dtype=np.float32))
    Wr = np.asarray(Wr, dtype=np.float32)
    br = np.asarray(br, dtype=np.float32)
    W1 = np.ascontiguousarray(np.asarray(W1, dtype=np.float32))
    b1 = np.ascontiguousarray(np.asarray(b1, dtype=np.float32))
    W2 = np.ascontiguousarray(np.asarray(W2, dtype=np.float32))
    b2 = np.ascontiguousarray(np.asarray(b2, dtype=np.float32))

    B, S, H = x.shape
    ntok = B * S
    xf = x.reshape(ntok, H)

    topw, topi = _route_host(x, Wr, br)
    topw = topw.reshape(ntok, TOP_K)
    topi = topi.reshape(ntok, TOP_K)

    # per-expert token index lists + combine weights
    idx = []
    wgt = []
    for e in range(NUM_EXPERTS):
        mask = (topi == e)
        tok = np.nonzero(mask.any(axis=1))[0]
        w = (topw * mask).sum(axis=1)[tok].astype(np.float32)
        idx.append(tok)
        wgt.append(w)
    counts = np.array([len(t) for t in idx])

    blk = int(os.environ.get("MOE_BLK", "512"))
    C = max(_round_up(int(counts.max()), 2), 512)

    key = (C, blk)
    if key not in _PROGRAM_CACHE:
        _PROGRAM_CACHE[key] = _build_program(C, blk)
    nc = _PROGRAM_CACHE[key]

    in_maps = []
    for e in range(NUM_EXPERTS):
        cnt = counts[e]
        xpe = np.zeros((C, H), dtype=np.float32)
        xpe[:cnt] = xf[idx[e]]
        # xp[p, hc, t] = x_t[hc*128+p]
        xpe = np.ascontiguousarray(
            xpe.T.reshape(HC, 128, C).transpose(1, 0, 2).astype(bf16))
        scle = np.zeros((C,), dtype=np.float32)
        scle[:cnt] = wgt[e]
        # w1p[p, fc, hc, j] = W1[e][hc*128+p, fc*128+j]
        w1p = np.ascontiguousarray(
            W1[e].reshape(HC, 128, FC, 128).transpose(1, 2, 0, 3).astype(bf16))
        # w2p[p, oc, fc, j] = W2[e][fc*128+p, oc*128+j]
        w2p = np.ascontiguousarray(
            W2[e].reshape(FC, 128, HC, 128).transpose(1, 2, 0, 3).astype(bf16))
        in_maps.append({
            "xp": xpe,
            "w1p": w1p,
            "b1": np.ascontiguousarray(b1[e]),
            "w2p": w2p,
            "b2": np.ascontiguousarray(b2[e]),
            "scl": scle,
        })

    trace = os.environ.get("MOE_TRACE", "0") == "1"
    res = run_bass_kernel_spmd(
        nc, in_maps, core_ids=list(range(NCORES)), trace=trace)
    LAST_EXEC_NS = res.exec_time_ns
    LAST_RESULTS = res

    out = np.zeros((ntok, H), dtype=np.float32)
    for e in range(NUM_EXPERTS):
        cnt = counts[e]
        ye = res.results[e]["yT"]  # [128, HC, C] f32
        ye = ye.transpose(1, 0, 2).reshape(H, C)[:, :cnt].T  # [cnt, H]
        out[idx[e]] += ye
    return out.reshape(B, S, H)


# revision 4
# speedup vs baseline: 1.0694x; 1.0150x over previous
"""MoE layer (8 experts, top-2) for 8 Trainium2 NeuronCores.

Strategy: expert-parallel. The router (0.1% of FLOPs) runs on host and
decides the sharding: tokens are gathered by routed expert host-side (the
all-to-all), each core runs one expert's dense MLP
  y = scale * (gelu(x @ W1 + b1) @ W2 + b2)
over the tokens routed to it, and the host scatter-adds the per-expert
partial outputs back.

Kernel structure (per core): all tensors bf16 on the wire (PSUM accumulates
f32), weight-stationary passes over the full token range. Tokens are split
into NB <= 8 blocks of <= 512 (one PSUM bank each). Stage B runs 16
f-chunk passes (8 k-steps x NB blocks, one LDWEIGHTS-worth of weights per
k-step held across all blocks); stage C runs 8 output-chunk passes (16
k-steps x NB blocks). Weight/x DMAs are ordered so the first matmul only
waits on ~0.8MB.
"""

import os

import numpy as np

HIDDEN = 1024
FF = 2 * HIDDEN
NUM_EXPERTS = 8
TOP_K = 2
NCORES = 8
HC = HIDDEN // 128   # 8 k-chunks for stage B / out-chunks for stage C
FC = FF // 128       # 16 f-chunks

# Set by kernel() when MOE_TRACE=1: HW kernel execution time in ns.
LAST_EXEC_NS = None
LAST_RESULTS = None

_PROGRAM_CACHE = {}


def _round_up(v, m):
    return (v + m - 1) // m * m


def _blocks_of(C, blk):
    """Split C tokens into blocks of <=blk, avoiding tiny tails."""
    blocks = []
    t0 = 0
    rem = C
    while rem > 0:
        if rem >= blk + 256 or rem <= blk:
            b = min(blk, rem)
        else:
            b = rem - 256
        if b < 256 and rem > b:
            b = rem
        blocks.append((t0, b))
        t0 += b
        rem -= b
    return blocks


def _build_program(C, blk):
    import concourse.bass as bass  # noqa: F401
    import concourse.mybir as mybir
    import concourse.tile as tile
    from concourse import bacc

    f32 = mybir.dt.float32
    bf16 = mybir.dt.bfloat16

    nc = bacc.Bacc("TRN2", target_bir_lowering=False, debug=False,
                   num_devices=NCORES)

    # DRAM layouts (host-packed, partition-major so every DMA reads >=2KB
    # contiguous per partition):
    #   xp [128, HC, C]       bf16: xp[p, hc, t] = x_t[hc*128+p]
    #   w1p[128, FC, HC, 128] bf16: w1p[p, fc, hc, j] = W1[hc*128+p, fc*128+j]
    #   w2p[128, OC, FC, 128] bf16: w2p[p, oc, fc, j] = W2[fc*128+p, oc*128+j]
    #   yT [128, OC, C]       f32:  y_t[oc*128+p] = yT[p, oc, t]
    xp = nc.dram_tensor("xp", [128, HC, C], bf16, kind="ExternalInput")
    w1 = nc.dram_tensor("w1p", [128, FC, HC, 128], bf16, kind="ExternalInput")
    b1 = nc.dram_tensor("b1", [FF], f32, kind="ExternalInput")
    w2 = nc.dram_tensor("w2p", [128, HC, FC, 128], bf16, kind="ExternalInput")
    b2 = nc.dram_tensor("b2", [HIDDEN], f32, kind="ExternalInput")
    scl = nc.dram_tensor("scl", [C], f32, kind="ExternalInput")
    yT = nc.dram_tensor("yT", [128, HC, C], f32, kind="ExternalOutput")

    blocks = _blocks_of(C, blk)
    NB = len(blocks)
    assert NB <= 8, f"need one PSUM bank per block, got {NB}"
    # Column halves for stage B: passes over half 0 start as soon as its x
    # columns land, hiding the DMA of half 1's x + remaining weights.
    nh0 = min(2, NB - 1) if NB > 1 else 1
    halves = [blocks[:nh0], blocks[nh0:]] if NB > 1 else [blocks]
    spans = [(h[0][0], h[-1][0] + h[-1][1]) for h in halves]

    Gelu = mybir.ActivationFunctionType.Gelu
    Ident = mybir.ActivationFunctionType.Identity

    with tile.TileContext(nc) as tc:
        with (
            tc.tile_pool(name="wts", bufs=1) as wts,
            tc.tile_pool(name="xin", bufs=1) as xin,
            tc.tile_pool(name="hmid", bufs=1) as hmid,
            tc.tile_pool(name="outs", bufs=4) as outs,
            tc.tile_pool(name="ps", bufs=8, space="PSUM") as ps,
        ):
            # --- DMA issue order on the sync ring: first matmul's deps
            # first (w1[fc=0], x half 0), then remaining w1 (needed one
            # fc-pass apart), then x half 1, then w2 (needed ~halfway).
            w1_sb = [None] * FC
            x_sb = [[None] * len(halves) for _ in range(HC)]

            def load_w1(fc):
                t = wts.tile([128, HC, 128], bf16, tag=f"w1f{fc}", name=f"w1f{fc}")
                nc.sync.dma_start(out=t[:], in_=w1.ap()[:, fc])
                w1_sb[fc] = t

            def load_x(hc, hi):
                c0, c1 = spans[hi]
                t = xin.tile([128, c1 - c0], bf16, tag=f"x{hc}_{hi}",
                             name=f"x{hc}_{hi}")
                nc.sync.dma_start(out=t[:], in_=xp.ap()[:, hc, c0:c1])
                x_sb[hc][hi] = t

            def xs(hc, t0, bs):
                # slice of x for block starting at t0 (inside one half)
                for hi, (c0, c1) in enumerate(spans):
                    if c0 <= t0 < c1:
                        return x_sb[hc][hi][:, t0 - c0:t0 - c0 + bs]
                raise AssertionError

            load_w1(0)
            for hc in range(HC):
                load_x(hc, 0)
            for fc in range(1, FC):
                load_w1(fc)
            if len(halves) > 1:
                for hc in range(HC):
                    load_x(hc, 1)
            w2_sb = []
            for oc in range(HC):
                t = wts.tile([128, FC, 128], bf16, tag=f"w2o{oc}", name=f"w2o{oc}")
                nc.sync.dma_start(out=t[:], in_=w2.ap()[:, oc])
                w2_sb.append(t)

            # small stuff on the scalar ring
            b1_sb = wts.tile([128, FC], f32, tag="b1")
            nc.scalar.dma_start(
                out=b1_sb[:], in_=b1.ap().rearrange("(c p) -> p c", p=128))
            b2_sb = wts.tile([128, HC], f32, tag="b2")
            nc.scalar.dma_start(
                out=b2_sb[:], in_=b2.ap().rearrange("(c p) -> p c", p=128))
            s_sb = wts.tile([128, C], f32, tag="scl")
            nc.scalar.dma_start(
                out=s_sb[:], in_=scl.ap().partition_broadcast(128))

            h_sb = [hmid.tile([128, C], bf16, tag=f"h{fc}", name=f"h{fc}")
                    for fc in range(FC)]

            # --- Stage B: h[fc] = gelu(sum_hc w1[hc,fc].T @ x[hc] + b1[fc])
            for hi, hblocks in enumerate(halves):
                for fc in range(FC):
                    phs = [ps.tile([128, blk], f32, tag="ps",
                                   name=f"psB{hi}_{fc}_{b}")
                           for b in range(len(hblocks))]
                    for hc in range(HC):
                        lhsT = w1_sb[fc][:, hc]
                        for b, (t0, bs) in enumerate(hblocks):
                            nc.tensor.matmul(
                                phs[b][:, :bs],
                                lhsT,
                                xs(hc, t0, bs),
                                start=(hc == 0), stop=(hc == HC - 1),
                            )
                    for b, (t0, bs) in enumerate(hblocks):
                        nc.scalar.activation(
                            out=h_sb[fc][:, t0:t0 + bs], in_=phs[b][:, :bs],
                            func=Gelu, bias=b1_sb[:, fc:fc + 1], scale=1.0)

            # --- Stage C: y[oc] = scl * (sum_fc w2[fc,oc].T @ h[fc] + b2[oc])
            # Block-outer so each block's epilogue (act -> mul -> DMA out,
            # on scalar/vector/gpsimd) pipelines under the next block's
            # matmul stream; only the last block's epilogue is exposed.
            for oc in range(HC):
                for b, (t0, bs) in enumerate(blocks):
                    py = ps.tile([128, blk], f32, tag="ps", name=f"psC{oc}_{b}")
                    for fc in range(FC):
                        nc.tensor.matmul(
                            py[:, :bs],
                            w2_sb[oc][:, fc],
                            h_sb[fc][:, t0:t0 + bs],
                            start=(fc == 0), stop=(fc == FC - 1),
                        )
                    o1 = outs.tile([128, blk], f32, tag="o1", name=f"o{oc}_{b}")
                    nc.scalar.activation(
                        out=o1[:, :bs], in_=py[:, :bs], func=Ident,
                        bias=b2_sb[:, oc:oc + 1], scale=1.0)
                    nc.vector.tensor_mul(
                        o1[:, :bs], o1[:, :bs], s_sb[:, t0:t0 + bs])
                    nc.gpsimd.dma_start(
                        out=yT.ap()[:, oc, t0:t0 + bs], in_=o1[:, :bs])

    nc.compile()
    return nc


def _route_host(x, Wr, br):
    """Replicate the reference router bit-exactly (jax on CPU), with a
    numpy fallback (same math, same tie semantics) if jax-cpu is absent."""
    try:
        import jax
        import jax.numpy as jnp

        cpu = jax.devices("cpu")[0]
        xj = jax.device_put(x, cpu)
        Wrj = jax.device_put(Wr, cpu)
        brj = jax.device_put(br, cpu)
        with jax.default_device(cpu):
            logits = jnp.einsum("bsh,he->bse", xj, Wrj) + brj
            routing = jax.nn.softmax(logits, axis=-1)
            topw, topi = jax.lax.top_k(routing, TOP_K)
            topw = jax.nn.softmax(topw, axis=-1)
        return np.asarray(topw), np.asarray(topi)
    except Exception:
        lg = x.reshape(-1, x.shape[-1]).astype(np.float32) @ Wr + br
        m = lg.max(axis=-1, keepdims=True)
        p = np.exp(lg - m)
        p /= p.sum(axis=-1, keepdims=True)
        # top-k with lower-index-wins tie semantics (jax.lax.top_k)
        topi = np.argsort(-p, axis=-1, kind="stable")[:, :TOP_K]
        topv = np.take_along_axis(p, topi, axis=-1)
        e = np.exp(topv - topv.max(axis=-1, keepdims=True))
        topw = (e / e.sum(axis=-1, keepdims=True)).astype(np.float32)
        B, S = x.shape[0], x.shape[1]
        return (topw.reshape(B, S, TOP_K),
                topi.astype(np.int32).reshape(B, S, TOP_K))


def kernel(x, Wr, br, W1, b1, W2, b2):
    global LAST_EXEC_NS, LAST_RESULTS
    import ml_dtypes
    from concourse.bass_utils import run_bass_kernel_spmd

    bf16 = ml_dtypes.bfloat16

    x = np.ascontiguousarray(np.asarray(x, dtype=np.float32))
    Wr = np.asarray(Wr, dtype=np.float32)
    br = np.asarray(br, dtype=np.float32)
    W1 = np.ascontiguousarray(np.asarray(W1, dtype=np.float32))
    b1 = np.ascontiguousarray(np.asarray(b1, dtype=np.float32))
    W2 = np.ascontiguousarray(np.asarray(W2, dtype=np.float32))
    b2 = np.ascontiguousarray(np.asarray(b2, dtype=np.float32))

    B, S, H = x.shape
    ntok = B * S
    xf = x.reshape(ntok, H)

    topw, topi = _route_host(x, Wr, br)
    topw = topw.reshape(ntok, TOP_K)
    topi = topi.reshape(ntok, TOP_K)

    # per-expert token index lists + combine weights
    idx = []
    wgt = []
    for e in range(NUM_EXPERTS):
        mask = (topi == e)
        tok = np.nonzero(mask.any(axis=1))[0]
        w = (topw * mask).sum(axis=1)[tok].astype(np.float32)
        idx.append(tok)
        wgt.append(w)
    counts = np.array([len(t) for t in idx])

    blk = int(os.environ.get("MOE_BLK", "512"))
    C = max(_round_up(int(counts.max()), 2), 512)

    key = (C, blk)
    if key not in _PROGRAM_CACHE:
        _PROGRAM_CACHE[key] = _build_program(C, blk)
    nc = _PROGRAM_CACHE[key]

    in_maps = []
    for e in range(NUM_EXPERTS):
        cnt = counts[e]
        xpe = np.zeros((C, H), dtype=np.float32)
        xpe[:cnt] = xf[idx[e]]
        # xp[p, hc, t] = x_t[hc*128+p]
        xpe = np.ascontiguousarray(
            xpe.T.reshape(HC, 128, C).transpose(1, 0, 2).astype(bf16))
        scle = np.zeros((C,), dtype=np.float32)
        scle[:cnt] = wgt[e]
        # w1p[p, fc, hc, j] = W1[e][hc*128+p, fc*128+j]
        w1p = np.ascontiguousarray(
            W1[e].reshape(HC, 128, FC, 128).transpose(1, 2, 0, 3).astype(bf16))
        # w2p[p, oc, fc, j] = W2[e][fc*128+p, oc*128+j]
        w2p = np.ascontiguousarray(
            W2[e].reshape(FC, 128, HC, 128).transpose(1, 2, 0, 3).astype(bf16))
        in_maps.append({
            "xp": xpe,
            "w1p": w1p,
            "b1": np.ascontiguousarray(b1[e]),
            "w2p": w2p,
            "b2": np.ascontiguousarray(b2[e]),
            "scl": scle,
        })

    trace = os.environ.get("MOE_TRACE", "0") == "1"
    res = run_bass_kernel_spmd(
        nc, in_maps, core_ids=list(range(NCORES)), trace=trace)
    LAST_EXEC_NS = res.exec_time_ns
    LAST_RESULTS = res

    out = np.zeros((ntok, H), dtype=np.float32)
    for e in range(NUM_EXPERTS):
        cnt = counts[e]
        ye = res.results[e]["yT"]  # [128, HC, C] f32
        ye = ye.transpose(1, 0, 2).reshape(H, C)[:, :cnt].T  # [cnt, H]
        out[idx[e]] += ye
    return out.reshape(B, S, H)
